# revision 6
# baseline (speedup 1.0000x reference)
"""DistMult edge scoring on 8 Trainium2 NeuronCores.

score[e] = sum_d node_emb[src[e], d] * rel_emb[e, d] * node_emb[dst[e], d]

Strategy (data-parallel over edges, per the sharding hint):
  - Edges sharded contiguously across 8 cores (125k/core, padded to whole
    128x32 tiles). Node table replicated per core in HBM.
  - Gather: gpsimd dma_gather is descriptor-generation-bound on the Q7
    DSP cores (~8 ns/descriptor measured). The ucode gates descgen with
    `cpu_id / 2 == queue_num`, so gathers on different SWDGE queue_nums
    generate descriptors CONCURRENTLY on different Q7 core pairs. Each
    tile's two directions are split into four 2048-index gathers on
    queues 0-3 -> ~4x the descriptor throughput of the single-queue
    baseline (2.05 ms Q7-busy -> ~0.5 ms).
  - int16 gather indices cap the addressable range at 32767, so the table
    is viewed as 25000 groups of 4 rows (1 KB each): one descriptor
    fetches an edge's whole 4-row group (group id = node >> 2 < 25000
    fits int16), and DVE selects the right row with host-precomputed 0/1
    masks (node & 3). One descriptor per edge per direction.
  - Raw bacc with a manually pipelined 2-slot schedule.
"""

import numpy as np

import concourse.bacc as bacc
import concourse.bass as bass
import concourse.mybir as mybir
from concourse import library_config
from concourse.bass_utils import run_bass_kernel_spmd

N_NODES = 100000
DIM = 64
N_EDGES = 1000000
N_CORES = 8

P = 128
K = 32
TILE = P * K                      # 4096 edges per tile
EPC = N_EDGES // N_CORES          # 125000
NT = -(-EPC // TILE)              # 31
EPAD = NT * TILE                  # 126976

GR = 4                            # rows per gather group
NGRP = N_NODES // GR              # 25000 groups, fits int16
HALF = TILE // 2                  # 2048 idx per sub-gather
QH = HALF // P                    # 16 free slots per sub-gather
IDXH = HALF // 16                 # 128 int16s per partition per sub-gather

F32 = mybir.dt.float32

_cache = {}


def _half_valid(t, h):
    """Valid (non-pad) edge count in tile t, half h; pad idx are trailing
    -1s skipped by num_idxs_reg semantics. All-pad halves keep one valid
    index (host forces slot 0 to group 0) so the DMA completion fires."""
    lo = t * TILE + h * HALF
    return int(max(1, min(HALF, EPC - lo)))


def _build_program():
    if "nc" in _cache:
        return _cache["nc"]

    nc = bacc.Bacc(
        "TRN2",
        target_bir_lowering=False,
        debug=False,
        enable_asserts=False,
        num_devices=N_CORES,
        num_swdge_queues=4,
    )
    table = nc.dram_tensor("table", [NGRP, GR * DIM], F32, kind="ExternalInput")
    # idx[t, p, d, h, :]: sub-gather (d, h) of tile t, wrapped per 2048
    idx_h = nc.dram_tensor(
        "idx", [NT, P, 2, 2, IDXH], mybir.dt.int16, kind="ExternalInput"
    )
    rel_h = nc.dram_tensor("rel", [NT, P, K, DIM], F32, kind="ExternalInput")
    msk_h = nc.dram_tensor("msk", [NT, P, 2, GR, K], F32, kind="ExternalInput")
    out_h = nc.dram_tensor("score", [NT, P, K], F32, kind="ExternalOutput")

    NB = 2     # gather-buffer pipeline slots
    NB_IO = 4  # idx/rel/msk prefetch slots
    NOPS = 17  # DVE ops per tile

    from contextlib import ExitStack

    es = ExitStack()
    with es:
        block = es.enter_context(nc.Block())
        gbuf = es.enter_context(
            nc.sbuf_tensor("gbuf", [P, NB, 2, K, GR * DIM], F32))
        relb = es.enter_context(
            nc.sbuf_tensor("relb", [P, NB_IO, K, DIM], F32))
        mskb = es.enter_context(
            nc.sbuf_tensor("mskb", [P, NB_IO, 2, GR, K], F32))
        idxb = es.enter_context(
            nc.sbuf_tensor("idxb", [P, NB_IO, 2, 2, IDXH], mybir.dt.int16))
        hbuf = es.enter_context(nc.sbuf_tensor("hbuf", [P, K, DIM], F32))
        tbuf = es.enter_context(nc.sbuf_tensor("tbuf", [P, K, DIM], F32))
        tmpb = es.enter_context(nc.sbuf_tensor("tmpb", [P, K, DIM], F32))
        sb_s = es.enter_context(nc.sbuf_tensor("sbuf_s", [P, NB, K], F32))
        s_idx = es.enter_context(nc.semaphore("s_idx"))
        s_rel = es.enter_context(nc.semaphore("s_rel"))
        s_msk = es.enter_context(nc.semaphore("s_msk"))
        s_vc = es.enter_context(nc.semaphore("s_vc"))
        s_out = es.enter_context(nc.semaphore("s_out"))
        s_q = tuple(
            tuple(
                es.enter_context(nc.semaphore(f"s_q{q}{p}"))
                for p in ("a", "b")
            )
            for q in range(4)
        )

        def q_done(eng, q, t):
            # tile t's queue-q gather completed (parity sems: completions
            # on one sem are 2 tiles apart, so out-of-order DMA finishes
            # cannot be conflated)
            eng.wait_ge(s_q[q][t % 2], 16 * (t // 2 + 1))

        @block.sync
        def _(sp: bass.BassEngine):
            # pure prefetcher: never gated on the compute chain beyond
            # slot reuse (NB_IO slots deep)
            for t in range(NT):
                s = t % NB_IO
                if t >= 1:
                    # order completions: sem count N must imply tiles 0..N-1
                    # are actually resident (DMAs can finish out of order)
                    sp.wait_ge(s_idx, 16 * t)
                    sp.wait_ge(s_rel, 16 * t)
                    sp.wait_ge(s_msk, 16 * t)
                if t >= NB_IO:
                    # idx slot free once tile t-NB_IO's gathers retired
                    tt = t - NB_IO
                    for q in range(4):
                        q_done(sp, q, tt)
                sp.dma_start(out=idxb[:, s], in_=idx_h[t]).then_inc(s_idx, 16)
                if t >= NB_IO:
                    # rel/msk slots consumed by DVE of tile t-NB_IO
                    sp.wait_ge(s_vc, NOPS * (t - NB_IO + 1))
                sp.dma_start(out=relb[:, s], in_=rel_h[t]).then_inc(s_rel, 16)
                sp.dma_start(out=mskb[:, s], in_=msk_h[t]).then_inc(s_msk, 16)

        @block.scalar
        def _(sc: bass.BassEngine):
            # out-stores, decoupled from the prefetch stream
            for t in range(NT):
                sc.wait_ge(s_vc, NOPS * (t + 1))
                if t >= 1:
                    sc.wait_ge(s_out, 16 * t)
                sc.dma_start(
                    out=out_h[t], in_=sb_s[:, t % NB]
                ).then_inc(s_out, 16)
            sc.wait_ge(s_out, 16 * NT)

        @block.gpsimd
        def _(gp: bass.BassGpSimd):
            gp.load_library(library_config.mlp)
            for t in range(NT):
                s = t % NB
                gp.wait_ge(s_idx, 16 * (t + 1))
                if t >= NB:
                    # gather buffers of tile t-NB consumed by DVE
                    gp.wait_ge(s_vc, NOPS * (t - NB + 1))
                for d in range(2):
                    for h in range(2):
                        q = 2 * d + h
                        gp.dma_gather(
                            gbuf[:, s, d, h * QH : (h + 1) * QH],
                            table[:],
                            idxb[:, t % NB_IO, d, h],
                            HALF,
                            _half_valid(t, h),
                            GR * DIM,
                            elem_step=GR * DIM,
                            single_packet=False,
                            queue_num=q,
                        ).then_inc(s_q[q][t % 2], 16)

        @block.vector
        def _(v: bass.BassEngine):
            mult = mybir.AluOpType.mult
            add = mybir.AluOpType.add
            for t in range(NT):
                s = t % NB
                q_done(v, 0, t)  # src half 0 landed
                q_done(v, 1, t)  # src half 1 landed
                v.wait_ge(s_rel, 16 * (t + 1))
                v.wait_ge(s_msk, 16 * (t + 1))
                if t >= NB:
                    v.wait_ge(s_out, 16 * (t - NB + 1))
                if t >= 1:
                    # hbuf/tbuf/tmpb WAR vs previous tile's chain
                    v.wait_ge(s_vc, NOPS * t)
                # last tile: only ceil(valid/P) k-slots hold real edges
                KV = K if t < NT - 1 else -(-(EPC - t * TILE) // P)
                i = NOPS * t

                def op(instr):
                    nonlocal i
                    i += 1
                    instr.then_inc(s_vc, 1)

                def wait():
                    v.wait_ge(s_vc, i)

                for d, dst in ((0, hbuf), (1, tbuf)):
                    if d == 1:
                        q_done(v, 2, t)
                        q_done(v, 3, t)
                    g = gbuf[:, s, d]
                    for r in range(GR):
                        m = mskb[:, t % NB_IO, d, r, :KV].to_broadcast(
                            [P, KV, DIM]
                        )
                        gsl = g[:, :KV, r * DIM : (r + 1) * DIM]
                        if r == 0:
                            op(
                                v.tensor_tensor(
                                    out=dst[:, :KV], in0=gsl, in1=m, op=mult
                                )
                            )
                        else:
                            wait()
                            op(
                                v.tensor_tensor(
                                    out=tmpb[:, :KV], in0=gsl, in1=m, op=mult
                                )
                            )
                            wait()
                            op(
                                v.tensor_tensor(
                                    out=dst[:, :KV],
                                    in0=dst[:, :KV],
                                    in1=tmpb[:, :KV],
                                    op=add,
                                )
                            )
                wait()
                op(
                    v.tensor_tensor(
                        out=hbuf[:, :KV],
                        in0=hbuf[:, :KV],
                        in1=relb[:, t % NB_IO, :KV],
                        op=mult,
                    )
                )
                wait()
                op(
                    v.tensor_tensor(
                        out=hbuf[:, :KV], in0=hbuf[:, :KV], in1=tbuf[:, :KV],
                        op=mult,
                    )
                )
                wait()
                v.tensor_reduce(
                    out=sb_s[:, s, :KV],
                    in_=hbuf[:, :KV],
                    axis=mybir.AxisListType.X,
                    op=add,
                ).then_inc(s_vc, 1)

    nc.compile()
    _cache["nc"] = nc
    return nc


def _prep_idx(idx_global):
    """(EPAD,) node ids -> wrapped int16 group indices [NT, 2, P, IDXH].
    Pad edges (index < 0) stay -1 and are skipped via num_idxs_reg."""
    n = idx_global.reshape(NT, 2, HALF)
    g = np.where(n >= 0, n >> 2, -1).astype(np.int16)
    for t in range(NT):
        for h in range(2):
            if t * TILE + h * HALF >= EPC:
                g[t, h, 0] = 0  # keep >=1 valid idx per sub-gather
    # wrap: index j -> [j % 16, j // 16], replicated across 8 partition groups
    wr = g.reshape(NT, 2, IDXH, 16).swapaxes(2, 3)  # [NT, 2, 16, IDXH]
    return np.broadcast_to(
        wr[:, :, None, :, :], (NT, 2, 8, 16, IDXH)
    ).reshape(NT, 2, P, IDXH)


def _prep_msk(idx_global):
    """(EPAD,) node ids -> 0/1 row-select masks [NT, P, GR, K]."""
    sub = (np.maximum(idx_global.reshape(NT, K, P), 0) & 3).astype(np.int8)
    m = (sub[:, None, :, :] == np.arange(GR, dtype=np.int8)[None, :, None, None])
    # [NT, GR, K, P] -> [NT, P, GR, K]
    return np.ascontiguousarray(m.transpose(0, 3, 1, 2)).astype(np.float32)


def _shard_inputs(node_emb, rel_emb, src, dst):
    node_emb = np.asarray(node_emb, dtype=np.float32)
    rel_emb = np.asarray(rel_emb, dtype=np.float32)
    src = np.asarray(src, dtype=np.int64)
    dst = np.asarray(dst, dtype=np.int64)

    table = np.ascontiguousarray(node_emb.reshape(NGRP, GR * DIM))

    in_maps = []
    for c in range(N_CORES):
        sl = slice(c * EPC, (c + 1) * EPC)
        src_c = np.full(EPAD, -1, np.int64)
        dst_c = np.full(EPAD, -1, np.int64)
        rel_c = np.zeros((EPAD, DIM), np.float32)
        src_c[:EPC] = src[sl]
        dst_c[:EPC] = dst[sl]
        rel_c[:EPC] = rel_emb[sl]

        # [NT, 2dir, 2half, P, IDXH] -> [NT, P, 2dir, 2half, IDXH]
        idx = np.stack(
            [_prep_idx(src_c), _prep_idx(dst_c)], axis=1
        ).transpose(0, 3, 1, 2, 4)
        msk = np.stack([_prep_msk(src_c), _prep_msk(dst_c)], axis=2)
        # [NT, P, 2, GR, K]
        # edge j at [p = j % 128, k = j // 128] -> rel[t, p, k]
        rel_t = np.ascontiguousarray(
            rel_c.reshape(NT, K, P, DIM).swapaxes(1, 2)
        )
        in_maps.append(
            {
                "table": table,
                "idx": np.ascontiguousarray(idx),
                "rel": rel_t,
                "msk": msk,
            }
        )
    return in_maps


def run_on_hw(node_emb, rel_emb, src, dst, **spmd_kwargs):
    nc = _build_program()
    in_maps = _shard_inputs(node_emb, rel_emb, src, dst)
    res = run_bass_kernel_spmd(nc, in_maps, list(range(N_CORES)), **spmd_kwargs)
    parts = [
        np.asarray(res.results[c]["score"])
        .transpose(0, 2, 1)
        .reshape(EPAD)[:EPC]
        for c in range(N_CORES)
    ]
    return np.concatenate(parts), res


def kernel(node_emb, rel_emb, src, dst):
    scores, _ = run_on_hw(node_emb, rel_emb, src, dst)
    return scores
